# revision 26
# baseline (speedup 1.0000x reference)
"""Trainium2 Bass kernel for nn_MultiHeadAttention_8546984919667.

B=1, S=4096, D_MODEL=1024, H=16 heads, Dk=64.
Sharding: tensor-parallel over heads - each of the 8 cores owns 2 heads
(a 128-wide slice of the q/k/v projection outputs and of Wo's columns),
computes full attention for those heads, and produces a partial output
projection [S, D].  Host sums the 8 partials and adds bo.

v2 design: the whole kernel runs in 64x128 row-tiled PE mode (Dk=64
contractions on tiles T0 = SBUF partitions 0-63 / T8 = 64-127, which the
HW executes concurrently), so the Dk=64 attention matmuls stream at the
full PE rate instead of half:
  - scores: per 128-key chunk, h0 and h1 scores ([128t, 512s] each) are
    two concurrent K=64 matmuls into the two banks of one PSUM tile;
    one Exp [128, 1024] covers both heads (ACT is the roofline engine:
    256 exps * ~1.02us = ~262us).
  - ctx^T: per chunk, per head, two concurrent K=64 half-chunk matmuls
    accumulate into two PSUM banks (A: keys 0-63, B: 64-127); V is
    stored naturally (VnA layout [V_h0 | ones | V_h1] per chunk, the
    shared ones column gives both heads' softmax denominators via the
    aug trick).  A+=B on DVE at block end, then fast-reciprocal,
    broadcast via a tiny e01 matmul, normalize on evacuation.
  - projections (K/Q per-block/V-natural) and the output projection are
    also emitted as row-tiled half pairs, evacuated with one
    scalar_tensor_tensor (A + bias) + B, so the PE never mode-switches.
Software pipelining: scores run one chunk ahead of exp; ctx lags two
chunks (so PE never blocks in-order on ACT); V-natural production and
the K-proj tail overlap block 0; Q projections and the output
projection of block sb-1 fill PE slack inside block sb's t-loop.
"""

import sys

if "/opt/trn_rl_repo" not in sys.path:
    sys.path.insert(0, "/opt/trn_rl_repo")

import numpy as np
import ml_dtypes

import concourse.bass as bass
import concourse.tile as tile
from concourse import mybir
from concourse.bass_utils import run_bass_kernel_spmd

BF16 = ml_dtypes.bfloat16
F32 = mybir.dt.float32
BF = mybir.dt.bfloat16
ADD = mybir.AluOpType.add

S = 4096          # sequence length
D = 1024          # d_model
N_CORES = 8
DK = 64           # head dim
PC = 128          # projection slice per core (2 heads * DK)
NCH = D // 128    # 8 contraction chunks of 128 over d_model
SB = 512          # query-block width (PSUM bank)
NSB = S // SB     # 8 query blocks
NT = S // 128     # 32 key/value chunks of 128
AUG = DK + 1      # V block width with ones column
VW = 2 * DK + 2   # VnA per-chunk width: [V_h0 | ones | V_h1 | ones]
GW = 1024         # K-proj / input-tile column-group width
NG = S // GW      # 4 column groups

LAST_RESULT = None  # test harness reads exec_time_ns from here


def _split_multi_waits(nc):
    """This walrus build allows only one sync wait per instruction; move
    extras onto preceding same-engine NoOps."""
    for fn in nc.m.functions:
        for blk in fn.blocks:
            new_insts = []
            for ins in blk.instructions:
                si = ins.sync_info
                if si is not None and si.on_wait and len(si.on_wait) > 1:
                    extra = list(si.on_wait[:-1])
                    si.on_wait = [si.on_wait[-1]]
                    for j, w in enumerate(extra):
                        new_insts.append(mybir.InstNoOp(
                            name=f"{ins.name}-wsplit{j}",
                            engine=ins.engine,
                            ins=[], outs=[],
                            sync_info=mybir.SyncInfo(on_wait=[w], on_update=[]),
                        ))
                new_insts.append(ins)
            blk.instructions = new_insts


def _build():
    nc = bass.Bass("TRN2", target_bir_lowering=False, debug=False,
                   num_devices=N_CORES)

    qT = nc.dram_tensor("qT", [D, S], BF, kind="ExternalInput").ap()
    kT = nc.dram_tensor("kT", [D, S], BF, kind="ExternalInput").ap()
    vT = nc.dram_tensor("vT", [D, S], BF, kind="ExternalInput").ap()
    wq = nc.dram_tensor("wq", [D, PC], BF, kind="ExternalInput").ap()
    wk = nc.dram_tensor("wk", [D, PC], BF, kind="ExternalInput").ap()
    wv = nc.dram_tensor("wv", [D, PC], BF, kind="ExternalInput").ap()
    bqd = nc.dram_tensor("bqd", [PC, 1], F32, kind="ExternalInput").ap()
    bkd = nc.dram_tensor("bkd", [PC, 1], F32, kind="ExternalInput").ap()
    bvr = nc.dram_tensor("bvr", [1, PC], BF, kind="ExternalInput").ap()
    wo = nc.dram_tensor("wo", [PC, D], BF, kind="ExternalInput").ap()
    out = nc.dram_tensor("out", [S, D], F32, kind="ExternalOutput").ap()

    ts = bass.ts

    with tile.TileContext(nc) as tc:
        with (
            tc.tile_pool(name="persist", bufs=1) as persist,
            tc.tile_pool(name="xin", bufs=1) as xin,
            tc.tile_pool(name="ep", bufs=18) as ep,
            tc.tile_pool(name="scr", bufs=2) as scr,
            tc.tile_pool(name="cup", bufs=4) as cup,
            tc.tile_pool(name="cxp", bufs=2) as cxp,
            tc.tile_pool(name="otp", bufs=4) as otp,
            tc.tile_pool(name="ps", bufs=2, space="PSUM") as ps,
        ):
            # ---- persistent SBUF tensors ----
            KT = persist.tile([PC, S], BF, tag="KT")
            QT = persist.tile([PC, S], BF, tag="QT")
            VnA = persist.tile([128, NT * VW], BF, tag="VnA")
            w_q = persist.tile([128, D], BF, tag="w_q")
            w_k = persist.tile([128, D], BF, tag="w_k")
            w_v = persist.tile([128, D], BF, tag="w_v")
            w_o = persist.tile([PC, D], BF, tag="w_o")
            bq_s = persist.tile([PC, 1], F32, tag="bq_s")
            bk_s = persist.tile([PC, 1], F32, tag="bk_s")
            bvrow = persist.tile([DK, 128], BF, tag="bvrow")
            e0a = persist.tile([DK, 128], BF, tag="e0a")
            e01 = persist.tile([DK, 128], BF, tag="e01")
            rdenf = persist.tile([33, SB], F32, tag="rdenf")
            rdenb = persist.tile([DK, SB], BF, tag="rdenb")
            warm = persist.tile([128, 8], F32, tag="warm")

            # ---- load weights / constants ----
            for wtile, wdram in ((w_q, wq), (w_k, wk), (w_v, wv)):
                nc.sync.dma_start(
                    wtile[:].rearrange("p (c n) -> p c n", c=NCH),
                    wdram.rearrange("(c p) n -> p c n", c=NCH),
                )
            nc.sync.dma_start(w_o[:], wo[:, :])
            nc.sync.dma_start(bq_s[:], bqd[:, :])
            nc.sync.dma_start(bk_s[:], bkd[:, :])
            nc.gpsimd.memset(bvrow[:], 0.0)
            nc.sync.dma_start(bvrow[0:1, :], bvr[:, :])
            nc.gpsimd.memset(e0a[:], 0.0)
            nc.gpsimd.memset(e0a[0:1, :], 1.0)
            nc.gpsimd.memset(e01[:], 0.0)
            nc.gpsimd.memset(e01[0:1, 0:DK], 1.0)
            nc.gpsimd.memset(e01[32:33, DK:128], 1.0)
            nc.gpsimd.memset(rdenb[:], 0.0)
            # ones columns of VnA (cols DK and VW-1 of each chunk)
            nc.gpsimd.memset(
                VnA[:].rearrange("p (t a) -> p t a", a=VW)[:, :, DK:DK + 1],
                1.0)
            nc.gpsimd.memset(
                VnA[:].rearrange("p (t a) -> p t a", a=VW)[:, :, VW - 1:VW],
                1.0)
            # preload the ACT exp table early
            nc.gpsimd.memset(warm[:], 0.0)
            nc.scalar.activation(warm[:, 4:8], warm[:, 0:4],
                                 mybir.ActivationFunctionType.Exp, scale=1.0)

            with nc.allow_low_precision(reason="bf16 activations by design"):
                # ---- input tile pools / DMA ----
                # Q tiles for block pair p: 8 chunk-tiles [128, GW]
                def q_dma(p):
                    tiles = []
                    for c in range(NCH):
                        t = xin.tile([128, GW], BF, tag="qi", bufs=8,
                                     name=f"qi{p}_{c}")
                        nc.sync.dma_start(
                            t[:], qT[ts(c, 128), ts(p, GW)])
                        tiles.append(t)
                    return tiles

                def kv_dma(dram, g, who):
                    tiles = []
                    bufs = 16 if who == "ki" else 24
                    for c in range(NCH):
                        t = xin.tile([128, GW], BF, tag=who, bufs=bufs,
                                     name=f"{who}{g}_{c}")
                        nc.sync.dma_start(
                            t[:], dram[ts(c, 128), ts(g, GW)])
                        tiles.append(t)
                    return tiles

                # DMA emission order = HW transfer order (the input
                # stream is bandwidth-bound at ~410GB/s): K groups early
                # (scores cannot lag), V interleaved later (ctx can lag).
                qtiles = {0: q_dma(0)}
                ktiles, vtiles = [None] * NG, [None] * NG
                ktiles[0] = kv_dma(kT, 0, "ki")
                ktiles[1] = kv_dma(kT, 1, "ki")
                vtiles[0] = kv_dma(vT, 0, "vi")
                ktiles[2] = kv_dma(kT, 2, "ki")
                ktiles[3] = kv_dma(kT, 3, "ki")
                vtiles[1] = kv_dma(vT, 1, "vi")
                vtiles[2] = kv_dma(vT, 2, "vi")
                vtiles[3] = kv_dma(vT, 3, "vi")

                # ---- row-tiled projection helpers ----
                def qproj(sb, qh=None, width=SB):
                    """Project Q cols [sb*SB + qh*width, +width) into QT.
                    qh=None: whole block (prologue).  Atomic thunk: the PSUM
                    grab is produced and evacuated within one emission."""
                    qts = qtiles[sb // 2]
                    off = (sb % 2) * SB + (0 if qh is None else qh * width)
                    w = SB if qh is None else width
                    g = ps.tile([128, 2 * SB], F32, tag="sc",
                                name=f"qp{sb}_{qh}")
                    A, B = g[:, 0:w], g[:, SB:SB + w]
                    for c in range(NCH):
                        nc.tensor.matmul(A, w_q[0:DK, ts(c, 128)],
                                         qts[c][0:DK, off:off + w],
                                         start=(c == 0), stop=(c == NCH - 1))
                    for c in range(NCH):
                        nc.tensor.matmul(B, w_q[DK:PC, ts(c, 128)],
                                         qts[c][DK:PC, off:off + w],
                                         start=(c == 0), stop=(c == NCH - 1))
                    qtcol = sb * SB + (0 if qh is None else qh * width)
                    # DVE reads at most one PSUM operand: stage A in SBUF
                    q_s = scr.tile([128, 2 * SB], F32, tag="scr",
                                   name=f"qs{sb}_{qh}")
                    nc.vector.tensor_copy(q_s[:, 0:w], A)
                    nc.vector.scalar_tensor_tensor(
                        QT[:, qtcol:qtcol + w], q_s[:, 0:w], bq_s[:, 0:1], B,
                        op0=ADD, op1=ADD)

                def kproj(j):
                    """Project K column group j (512 wide) into KT.
                    Matmul PSUM output must stay within one bank (N<=512):
                    T0 half accumulates in bank a, T8 half in bank b."""
                    kts = ktiles[j // 2]
                    off = (j % 2) * SB
                    g = ps.tile([128, 2 * SB], F32, tag="sc", name=f"kp{j}")
                    A, B = g[:, 0:SB], g[:, SB:2 * SB]
                    for c in range(NCH):
                        nc.tensor.matmul(A, w_k[0:DK, ts(c, 128)],
                                         kts[c][0:DK, off:off + SB],
                                         start=(c == 0), stop=(c == NCH - 1))
                    for c in range(NCH):
                        nc.tensor.matmul(B, w_k[DK:PC, ts(c, 128)],
                                         kts[c][DK:PC, off:off + SB],
                                         start=(c == 0), stop=(c == NCH - 1))
                    k_s = scr.tile([128, 2 * SB], F32, tag="scr",
                                   name=f"ks{j}")
                    nc.vector.tensor_copy(k_s[:, 0:SB], A)
                    nc.vector.scalar_tensor_tensor(
                        KT[:, ts(j, SB)], k_s[:, 0:SB], bk_s[:, 0:1], B,
                        op0=ADD, op1=ADD)

                def vnat(tt):
                    """Produce natural-layout V chunk tt into VnA."""
                    g, col = tt // 8, (tt % 8) * 128
                    vts = vtiles[g]
                    grab = ps.tile([128, 2 * SB], F32, tag="sc",
                                   name=f"vn{tt}")
                    VA, VB = grab[:, 0:128], grab[:, SB:SB + 128]
                    # bias row: out[t, d] = bv[d] (e0a row0 = ones)
                    nc.tensor.matmul(VA, e0a[:, :], bvrow[:, :],
                                     start=True, stop=False)
                    for c in range(NCH):
                        nc.tensor.matmul(
                            VA, vts[c][0:DK, col:col + 128],
                            w_v[0:DK, ts(c, 128)],
                            start=False, stop=(c == NCH - 1))
                    for c in range(NCH):
                        nc.tensor.matmul(
                            VB, vts[c][DK:PC, col:col + 128],
                            w_v[DK:PC, ts(c, 128)],
                            start=(c == 0), stop=(c == NCH - 1))
                    base = tt * VW
                    v_s = scr.tile([128, 2 * SB], F32, tag="scr",
                                   name=f"vs{tt}")
                    nc.vector.tensor_copy(v_s[:, 0:128], VA)
                    nc.vector.tensor_add(
                        VnA[:, base:base + DK], v_s[:, 0:DK], VB[:, 0:DK])
                    nc.vector.tensor_add(
                        VnA[:, base + DK + 1:base + DK + 1 + DK],
                        v_s[:, DK:PC], VB[:, DK:PC])

                # ---- prologue: Q(0), Q(1), K-proj group 0 ----
                qproj(0)
                qproj(1)
                kproj(0)

                # ---- main flat loop over (sb, tt) ----
                items = [(sb, tt) for sb in range(NSB) for tt in range(NT)]
                n_items = len(items)
                sc_of = {}
                et_of = {}
                ctx_of = {}      # sb -> (cA0, cB0, cA1, cB1)
                ctxT_of = {}     # sb -> normalized ctx^T tile
                ctx_queue = []   # (sched_iter, emit_fn)
                thunks = {}      # iter -> [fn]

                def at(i, fn):
                    thunks.setdefault(i, []).append(fn)

                def emit_sc(idx):
                    sb, tt = items[idx]
                    sc = ps.tile([128, 2 * SB], F32, tag="sc",
                                 name=f"sc{sb}_{tt}")
                    nc.tensor.matmul(sc[:, 0:SB],
                                     KT[0:DK, ts(tt, 128)],
                                     QT[0:DK, ts(sb, SB)],
                                     start=True, stop=True)
                    nc.tensor.matmul(sc[:, SB:2 * SB],
                                     KT[DK:PC, ts(tt, 128)],
                                     QT[DK:PC, ts(sb, SB)],
                                     start=True, stop=True)
                    sc_of[idx] = sc

                def emit_exp(idx):
                    sb, tt = items[idx]
                    et = ep.tile([128, 2 * SB], BF, tag="et",
                                 name=f"et{sb}_{tt}")
                    nc.scalar.activation(
                        et[:], sc_of.pop(idx)[:],
                        mybir.ActivationFunctionType.Exp, scale=0.125)
                    et_of[idx] = et

                def emit_ctx(idx, cur_iter):
                    sb, tt = items[idx]
                    if tt == 0:
                        ctx_of[sb] = tuple(
                            ps.tile([128, SB], F32, tag="ctx", bufs=4,
                                    name=f"c{n}_{sb}")
                            for n in ("A0", "B0", "A1", "B1"))
                    cA0, cB0, cA1, cB1 = ctx_of[sb]
                    et = et_of.pop(idx)
                    st_, sp_ = (tt == 0), (tt == NT - 1)
                    base = tt * VW
                    nc.tensor.matmul(cA0[0:AUG, :],
                                     VnA[0:DK, base:base + AUG],
                                     et[0:DK, 0:SB], start=st_, stop=sp_)
                    nc.tensor.matmul(cB0[0:AUG, :],
                                     VnA[DK:PC, base:base + AUG],
                                     et[DK:PC, 0:SB], start=st_, stop=sp_)
                    nc.tensor.matmul(cA1[0:AUG, :],
                                     VnA[0:DK, base + DK + 1:base + VW],
                                     et[0:DK, SB:2 * SB],
                                     start=st_, stop=sp_)
                    nc.tensor.matmul(cB1[0:AUG, :],
                                     VnA[DK:PC, base + DK + 1:base + VW],
                                     et[DK:PC, SB:2 * SB],
                                     start=st_, stop=sp_)
                    if sp_:
                        emit_epilogue(sb, cur_iter)

                def emit_epilogue(sb, cur_iter):
                    cA0, cB0, cA1, cB1 = ctx_of.pop(sb)
                    # combine halves into SBUF (frees the ctx PSUM ring for
                    # the next block after just these two DVE adds), then
                    # fast-reciprocal of the denominators
                    cu0 = cup.tile([AUG, SB], F32, tag="cu", bufs=4,
                                   name=f"cu0_{sb}")
                    cu1 = cup.tile([AUG, SB], F32, tag="cu", bufs=4,
                                   name=f"cu1_{sb}")
                    nc.vector.tensor_copy(cu0[:], cA0[0:AUG, :])
                    nc.vector.tensor_copy(cu1[:], cA1[0:AUG, :])
                    nc.vector.tensor_add(cu0[:], cu0[:], cB0[0:AUG, :])
                    nc.vector.tensor_add(cu1[:], cu1[:], cB1[0:AUG, :])
                    nc.vector.reciprocal(rdenf[0:1, :], cu0[DK:AUG, :])
                    nc.vector.reciprocal(rdenf[32:33, :], cu1[DK:AUG, :])
                    nc.vector.tensor_copy(rdenb[0:1, :], rdenf[0:1, :])
                    nc.vector.tensor_copy(rdenb[32:33, :], rdenf[32:33, :])

                    def norm():
                        bg = ps.tile([128, 2 * SB], F32, tag="sc",
                                     name=f"bps{sb}")
                        bpsv = bg[:, 0:SB]
                        nc.tensor.matmul(bpsv, e01[:, :], rdenb[:, :],
                                         start=True, stop=True)
                        ctxT = cxp.tile([128, SB], BF, tag="ctxT",
                                        name=f"ctxT{sb}")
                        nc.vector.tensor_mul(ctxT[0:DK, :], cu0[0:DK, :],
                                             bpsv[0:DK, :])
                        nc.vector.tensor_mul(ctxT[DK:PC, :], cu1[0:DK, :],
                                             bpsv[DK:PC, :])
                        ctxT_of[sb] = ctxT
                    at(cur_iter + 4, norm)

                    def po_piece(j, sb=sb):
                        st_c, nh = j // 2, j % 2
                        ctxT = ctxT_of[sb]
                        pg = ps.tile([128, 2 * SB], F32, tag="sc",
                                     name=f"po{sb}_{j}")
                        poA, poB = pg[:, 0:SB], pg[:, SB:2 * SB]
                        nc.tensor.matmul(poA,
                                         ctxT[0:DK, ts(st_c, 128)],
                                         w_o[0:DK, ts(nh, SB)],
                                         start=True, stop=True)
                        nc.tensor.matmul(poB,
                                         ctxT[DK:PC, ts(st_c, 128)],
                                         w_o[DK:PC, ts(nh, SB)],
                                         start=True, stop=True)
                        ot = otp.tile([128, SB], F32, tag="ot",
                                      name=f"ot{sb}_{j}")
                        p_s = scr.tile([128, 2 * SB], F32, tag="scr",
                                       name=f"pos{sb}_{j}")
                        nc.vector.tensor_copy(p_s[:, 0:SB], poA)
                        nc.vector.tensor_add(ot[:], p_s[:, 0:SB], poB)
                        nc.sync.dma_start(
                            out[sb * SB + st_c * 128:
                                sb * SB + (st_c + 1) * 128,
                                ts(nh, SB)], ot[:])
                    for j in range(8):
                        at(cur_iter + 6 + 3 * j, lambda j=j: po_piece(j))

                # schedule block-0 K-proj tail (group j feeds scores(4j)
                # at iter 4j-1; thunk iters track the DMA arrival pacing) +
                # V-natural production (v tiles land late: lag the thunks so
                # the in-order PE never blocks on a v DMA)
                KP_ITER = {1: 0, 2: 4, 3: 7, 4: 14, 5: 17, 6: 20, 7: 23}
                for j, it in KP_ITER.items():
                    at(it, lambda j=j: kproj(j))
                VN_ITER = {}
                for tt in range(NT):
                    g, k = tt // 8, tt % 8
                    VN_ITER[tt] = (10, 24, 29, 34)[g] + k
                    at(VN_ITER[tt], lambda tt=tt: vnat(tt))
                for p in (1, 2, 3):
                    # DMA for pair p early in block 2p-2, proj during 2p-1
                    # (quarter-width atomic thunks so the borrowed PSUM slot
                    # is held only ~1.8us and ACT never starves)
                    at((2 * p - 2) * NT + 1,
                       lambda p=p: qtiles.__setitem__(p, q_dma(p)))
                    at((2 * p - 1) * NT + 8,
                       lambda p=p: qproj(2 * p, 0, 256))
                    at((2 * p - 1) * NT + 11,
                       lambda p=p: qproj(2 * p, 1, 256))
                    at((2 * p - 1) * NT + 20,
                       lambda p=p: qproj(2 * p + 1, 0, 256))
                    at((2 * p - 1) * NT + 23,
                       lambda p=p: qproj(2 * p + 1, 1, 256))

                emit_sc(0)
                for i in range(n_items):
                    emit_exp(i)
                    if i + 1 < n_items:
                        emit_sc(i + 1)
                    for fn in thunks.pop(i, ()):
                        fn()
                    sb, tt = items[i]
                    if sb == 0:
                        sched = max(i + 6, VN_ITER[tt] + 2)
                    else:
                        sched = i + 4
                    if ctx_queue:
                        sched = max(sched, ctx_queue[-1][0])
                    ctx_queue.append((sched, i))
                    while ctx_queue and ctx_queue[0][0] <= i:
                        _, idx = ctx_queue.pop(0)
                        emit_ctx(idx, i)
                # drain remaining ctx + thunks
                i = n_items
                while ctx_queue or thunks:
                    for fn in thunks.pop(i, ()):
                        fn()
                    while ctx_queue and ctx_queue[0][0] <= i:
                        _, idx = ctx_queue.pop(0)
                        emit_ctx(idx, i)
                    i += 1
                    assert i < n_items + 200, "drain did not converge"

    return nc


_NC = None


def _get_nc():
    global _NC
    if _NC is None:
        _NC = _build()
        _split_multi_waits(_NC)
    return _NC


def kernel(q, k, v, Wq, bq, Wk, bk, Wv, bv, Wo, bo):
    global LAST_RESULT
    nc = _get_nc()

    q2, k2, v2 = (np.asarray(x, np.float32)[0] for x in (q, k, v))
    qTh = np.ascontiguousarray(q2.T).astype(BF16)
    kTh = np.ascontiguousarray(k2.T).astype(BF16)
    vTh = np.ascontiguousarray(v2.T).astype(BF16)

    in_maps = []
    for c in range(N_CORES):
        sl = slice(c * PC, (c + 1) * PC)
        in_maps.append({
            "qT": qTh, "kT": kTh, "vT": vTh,
            "wq": np.ascontiguousarray(np.asarray(Wq, np.float32)[sl].T).astype(BF16),
            "wk": np.ascontiguousarray(np.asarray(Wk, np.float32)[sl].T).astype(BF16),
            "wv": np.ascontiguousarray(np.asarray(Wv, np.float32)[sl].T).astype(BF16),
            "bqd": np.asarray(bq, np.float32)[sl].reshape(PC, 1).copy(),
            "bkd": np.asarray(bk, np.float32)[sl].reshape(PC, 1).copy(),
            "bvr": np.asarray(bv, np.float32)[sl].reshape(1, PC).astype(BF16),
            "wo": np.ascontiguousarray(np.asarray(Wo, np.float32)[:, sl].T).astype(BF16),
        })

    res = run_bass_kernel_spmd(nc, in_maps, core_ids=list(range(N_CORES)))
    LAST_RESULT = res

    acc = np.zeros((S, D), np.float32)
    for c in range(N_CORES):
        acc += res.results[c]["out"]
    acc += np.asarray(bo, np.float32)[None, :]
    return acc[None].astype(np.float32)


# revision 27
# speedup vs baseline: 1.0908x; 1.0908x over previous
"""Trainium2 Bass kernel for nn_MultiHeadAttention_8546984919667.

B=1, S=4096, D_MODEL=1024, H=16 heads, Dk=64.
Sharding: tensor-parallel over heads - each of the 8 cores owns 2 heads
(a 128-wide slice of the q/k/v projection outputs and of Wo's columns),
computes full attention for those heads, and produces a partial output
projection [S, D].  Host sums the 8 partials and adds bo.

v2 design: the whole kernel runs in 64x128 row-tiled PE mode (Dk=64
contractions on tiles T0 = SBUF partitions 0-63 / T8 = 64-127, which the
HW executes concurrently), so the Dk=64 attention matmuls stream at the
full PE rate instead of half:
  - scores: per 128-key chunk, h0 and h1 scores ([128t, 512s] each) are
    two concurrent K=64 matmuls into the two banks of one PSUM tile;
    one Exp [128, 1024] covers both heads (ACT is the roofline engine:
    256 exps * ~1.02us = ~262us).
  - ctx^T: per chunk, per head, two concurrent K=64 half-chunk matmuls
    accumulate into two PSUM banks (A: keys 0-63, B: 64-127); V is
    stored naturally (VnA layout [V_h0 | ones | V_h1] per chunk, the
    shared ones column gives both heads' softmax denominators via the
    aug trick).  A+=B on DVE at block end, then fast-reciprocal,
    broadcast via a tiny e01 matmul, normalize on evacuation.
  - projections (K/Q per-block/V-natural) and the output projection are
    also emitted as row-tiled half pairs, evacuated with one
    scalar_tensor_tensor (A + bias) + B, so the PE never mode-switches.
Software pipelining: scores run one chunk ahead of exp; ctx lags two
chunks (so PE never blocks in-order on ACT); V-natural production and
the K-proj tail overlap block 0; Q projections and the output
projection of block sb-1 fill PE slack inside block sb's t-loop.
"""

import sys

if "/opt/trn_rl_repo" not in sys.path:
    sys.path.insert(0, "/opt/trn_rl_repo")

import numpy as np
import ml_dtypes

import concourse.bass as bass
import concourse.tile as tile
from concourse import mybir
from concourse.bass_utils import run_bass_kernel_spmd

BF16 = ml_dtypes.bfloat16
F32 = mybir.dt.float32
BF = mybir.dt.bfloat16
ADD = mybir.AluOpType.add

S = 4096          # sequence length
D = 1024          # d_model
N_CORES = 8
DK = 64           # head dim
PC = 128          # projection slice per core (2 heads * DK)
NCH = D // 128    # 8 contraction chunks of 128 over d_model
SB = 512          # query-block width (PSUM bank)
NSB = S // SB     # 8 query blocks
NT = S // 128     # 32 key/value chunks of 128
AUG = DK + 1      # V block width with ones column
VW = 2 * DK + 2   # VnA per-chunk width: [V_h0 | ones | V_h1 | ones]
GW = 1024         # K-proj / input-tile column-group width
NG = S // GW      # 4 column groups

LAST_RESULT = None  # test harness reads exec_time_ns from here


def _split_multi_waits(nc):
    """This walrus build allows only one sync wait per instruction; move
    extras onto preceding same-engine NoOps."""
    for fn in nc.m.functions:
        for blk in fn.blocks:
            new_insts = []
            for ins in blk.instructions:
                si = ins.sync_info
                if si is not None and si.on_wait and len(si.on_wait) > 1:
                    extra = list(si.on_wait[:-1])
                    si.on_wait = [si.on_wait[-1]]
                    for j, w in enumerate(extra):
                        new_insts.append(mybir.InstNoOp(
                            name=f"{ins.name}-wsplit{j}",
                            engine=ins.engine,
                            ins=[], outs=[],
                            sync_info=mybir.SyncInfo(on_wait=[w], on_update=[]),
                        ))
                new_insts.append(ins)
            blk.instructions = new_insts


def _build():
    nc = bass.Bass("TRN2", target_bir_lowering=False, debug=False,
                   num_devices=N_CORES)

    qT = nc.dram_tensor("qT", [D, S], BF, kind="ExternalInput").ap()
    kT = nc.dram_tensor("kT", [D, S], BF, kind="ExternalInput").ap()
    vT = nc.dram_tensor("vT", [D, S], BF, kind="ExternalInput").ap()
    wq = nc.dram_tensor("wq", [D, PC], BF, kind="ExternalInput").ap()
    wk = nc.dram_tensor("wk", [D, PC], BF, kind="ExternalInput").ap()
    wv = nc.dram_tensor("wv", [D, PC], BF, kind="ExternalInput").ap()
    bqd = nc.dram_tensor("bqd", [PC, 1], F32, kind="ExternalInput").ap()
    bkd = nc.dram_tensor("bkd", [PC, 1], F32, kind="ExternalInput").ap()
    bvr = nc.dram_tensor("bvr", [1, PC], BF, kind="ExternalInput").ap()
    wo = nc.dram_tensor("wo", [PC, D], BF, kind="ExternalInput").ap()
    out = nc.dram_tensor("out", [S, D], F32, kind="ExternalOutput").ap()

    ts = bass.ts

    with tile.TileContext(nc) as tc:
        with (
            tc.tile_pool(name="persist", bufs=1) as persist,
            tc.tile_pool(name="xin", bufs=1) as xin,
            tc.tile_pool(name="ep", bufs=18) as ep,
            tc.tile_pool(name="scr", bufs=2) as scr,
            tc.tile_pool(name="cup", bufs=4) as cup,
            tc.tile_pool(name="cxp", bufs=2) as cxp,
            tc.tile_pool(name="otp", bufs=4) as otp,
            tc.tile_pool(name="ps", bufs=2, space="PSUM") as ps,
        ):
            # ---- persistent SBUF tensors ----
            KT = persist.tile([PC, S], BF, tag="KT")
            QT = persist.tile([PC, S], BF, tag="QT")
            VnA = persist.tile([128, NT * VW], BF, tag="VnA")
            w_q = persist.tile([128, D], BF, tag="w_q")
            w_k = persist.tile([128, D], BF, tag="w_k")
            w_v = persist.tile([128, D], BF, tag="w_v")
            w_o = persist.tile([PC, D], BF, tag="w_o")
            bq_s = persist.tile([PC, 1], F32, tag="bq_s")
            bk_s = persist.tile([PC, 1], F32, tag="bk_s")
            bvrow = persist.tile([DK, 128], BF, tag="bvrow")
            e0a = persist.tile([DK, 128], BF, tag="e0a")
            e01 = persist.tile([DK, 128], BF, tag="e01")
            rdenf = persist.tile([33, SB], F32, tag="rdenf")
            rdenb = persist.tile([DK, SB], BF, tag="rdenb")
            warm = persist.tile([128, 8], F32, tag="warm")

            # ---- load weights / constants ----
            for wtile, wdram in ((w_q, wq), (w_k, wk), (w_v, wv)):
                nc.gpsimd.dma_start(
                    wtile[:].rearrange("p (c n) -> p c n", c=NCH),
                    wdram.rearrange("(c p) n -> p c n", c=NCH),
                )
            nc.gpsimd.dma_start(w_o[:], wo[:, :])
            nc.gpsimd.dma_start(bq_s[:], bqd[:, :])
            nc.gpsimd.dma_start(bk_s[:], bkd[:, :])
            nc.gpsimd.memset(bvrow[:], 0.0)
            nc.gpsimd.dma_start(bvrow[0:1, :], bvr[:, :])
            nc.gpsimd.memset(e0a[:], 0.0)
            nc.gpsimd.memset(e0a[0:1, :], 1.0)
            nc.gpsimd.memset(e01[:], 0.0)
            nc.gpsimd.memset(e01[0:1, 0:DK], 1.0)
            nc.gpsimd.memset(e01[32:33, DK:128], 1.0)
            nc.gpsimd.memset(rdenb[:], 0.0)
            # ones columns of VnA (cols DK and VW-1 of each chunk)
            nc.gpsimd.memset(
                VnA[:].rearrange("p (t a) -> p t a", a=VW)[:, :, DK:DK + 1],
                1.0)
            nc.gpsimd.memset(
                VnA[:].rearrange("p (t a) -> p t a", a=VW)[:, :, VW - 1:VW],
                1.0)
            # preload the ACT exp table early
            nc.gpsimd.memset(warm[:], 0.0)
            nc.scalar.activation(warm[:, 4:8], warm[:, 0:4],
                                 mybir.ActivationFunctionType.Exp, scale=1.0)

            with nc.allow_low_precision(reason="bf16 activations by design"):
                # ---- input tile pools / DMA ----
                # Q tiles for block pair p: 8 chunk-tiles [128, GW]
                def q_dma(p):
                    tiles = []
                    for c in range(NCH):
                        t = xin.tile([128, GW], BF, tag="qi", bufs=8,
                                     name=f"qi{p}_{c}")
                        nc.sync.dma_start(
                            t[:], qT[ts(c, 128), ts(p, GW)])
                        tiles.append(t)
                    return tiles

                def kv_dma(dram, g, who):
                    tiles = []
                    bufs = 16 if who == "ki" else 24
                    for c in range(NCH):
                        t = xin.tile([128, GW], BF, tag=who, bufs=bufs,
                                     name=f"{who}{g}_{c}")
                        nc.sync.dma_start(
                            t[:], dram[ts(c, 128), ts(g, GW)])
                        tiles.append(t)
                    return tiles

                # DMA emission order = HW transfer order (the input
                # stream is bandwidth-bound at ~410GB/s): K groups early
                # (scores cannot lag), V interleaved later (ctx can lag).
                qtiles = {0: q_dma(0)}
                ktiles, vtiles = [None] * NG, [None] * NG
                ktiles[0] = kv_dma(kT, 0, "ki")
                ktiles[1] = kv_dma(kT, 1, "ki")
                vtiles[0] = kv_dma(vT, 0, "vi")
                ktiles[2] = kv_dma(kT, 2, "ki")
                ktiles[3] = kv_dma(kT, 3, "ki")
                vtiles[1] = kv_dma(vT, 1, "vi")
                vtiles[2] = kv_dma(vT, 2, "vi")
                vtiles[3] = kv_dma(vT, 3, "vi")

                # ---- row-tiled projection helpers ----
                def qproj(sb, qh=None, width=SB):
                    """Project Q cols [sb*SB + qh*width, +width) into QT.
                    qh=None: whole block (prologue).  Atomic thunk: the PSUM
                    grab is produced and evacuated within one emission."""
                    qts = qtiles[sb // 2]
                    off = (sb % 2) * SB + (0 if qh is None else qh * width)
                    w = SB if qh is None else width
                    g = ps.tile([128, 2 * SB], F32, tag="sc",
                                name=f"qp{sb}_{qh}")
                    A, B = g[:, 0:w], g[:, SB:SB + w]
                    for c in range(NCH):
                        nc.tensor.matmul(A, w_q[0:DK, ts(c, 128)],
                                         qts[c][0:DK, off:off + w],
                                         start=(c == 0), stop=(c == NCH - 1))
                    for c in range(NCH):
                        nc.tensor.matmul(B, w_q[DK:PC, ts(c, 128)],
                                         qts[c][DK:PC, off:off + w],
                                         start=(c == 0), stop=(c == NCH - 1))
                    qtcol = sb * SB + (0 if qh is None else qh * width)
                    # DVE reads at most one PSUM operand: stage A in SBUF
                    q_s = scr.tile([128, 2 * SB], F32, tag="scr",
                                   name=f"qs{sb}_{qh}")
                    nc.vector.tensor_copy(q_s[:, 0:w], A)
                    nc.vector.scalar_tensor_tensor(
                        QT[:, qtcol:qtcol + w], q_s[:, 0:w], bq_s[:, 0:1], B,
                        op0=ADD, op1=ADD)

                def kproj(j):
                    """Project K column group j (512 wide) into KT.
                    Matmul PSUM output must stay within one bank (N<=512):
                    T0 half accumulates in bank a, T8 half in bank b."""
                    kts = ktiles[j // 2]
                    off = (j % 2) * SB
                    g = ps.tile([128, 2 * SB], F32, tag="sc", name=f"kp{j}")
                    A, B = g[:, 0:SB], g[:, SB:2 * SB]
                    for c in range(NCH):
                        nc.tensor.matmul(A, w_k[0:DK, ts(c, 128)],
                                         kts[c][0:DK, off:off + SB],
                                         start=(c == 0), stop=(c == NCH - 1))
                    for c in range(NCH):
                        nc.tensor.matmul(B, w_k[DK:PC, ts(c, 128)],
                                         kts[c][DK:PC, off:off + SB],
                                         start=(c == 0), stop=(c == NCH - 1))
                    k_s = scr.tile([128, 2 * SB], F32, tag="scr",
                                   name=f"ks{j}")
                    nc.vector.tensor_copy(k_s[:, 0:SB], A)
                    nc.vector.scalar_tensor_tensor(
                        KT[:, ts(j, SB)], k_s[:, 0:SB], bk_s[:, 0:1], B,
                        op0=ADD, op1=ADD)

                def vnat(tt):
                    """Produce natural-layout V chunk tt into VnA."""
                    g, col = tt // 8, (tt % 8) * 128
                    vts = vtiles[g]
                    grab = ps.tile([128, 2 * SB], F32, tag="sc",
                                   name=f"vn{tt}")
                    VA, VB = grab[:, 0:128], grab[:, SB:SB + 128]
                    # bias row: out[t, d] = bv[d] (e0a row0 = ones)
                    nc.tensor.matmul(VA, e0a[:, :], bvrow[:, :],
                                     start=True, stop=False)
                    for c in range(NCH):
                        nc.tensor.matmul(
                            VA, vts[c][0:DK, col:col + 128],
                            w_v[0:DK, ts(c, 128)],
                            start=False, stop=(c == NCH - 1))
                    for c in range(NCH):
                        nc.tensor.matmul(
                            VB, vts[c][DK:PC, col:col + 128],
                            w_v[DK:PC, ts(c, 128)],
                            start=(c == 0), stop=(c == NCH - 1))
                    base = tt * VW
                    v_s = scr.tile([128, 2 * SB], F32, tag="scr",
                                   name=f"vs{tt}")
                    nc.vector.tensor_copy(v_s[:, 0:128], VA)
                    nc.vector.tensor_add(
                        VnA[:, base:base + DK], v_s[:, 0:DK], VB[:, 0:DK])
                    nc.vector.tensor_add(
                        VnA[:, base + DK + 1:base + DK + 1 + DK],
                        v_s[:, DK:PC], VB[:, DK:PC])

                # ---- prologue: Q(0), Q(1), K-proj group 0 ----
                qproj(0)
                qproj(1)
                kproj(0)

                # ---- main flat loop over (sb, tt) ----
                items = [(sb, tt) for sb in range(NSB) for tt in range(NT)]
                n_items = len(items)
                sc_of = {}
                et_of = {}
                ctx_of = {}      # sb -> (cA0, cB0, cA1, cB1)
                ctxT_of = {}     # sb -> normalized ctx^T tile
                ctx_queue = []   # (sched_iter, emit_fn)
                thunks = {}      # iter -> [fn]

                def at(i, fn):
                    thunks.setdefault(i, []).append(fn)

                def emit_sc(idx):
                    sb, tt = items[idx]
                    sc = ps.tile([128, 2 * SB], F32, tag="sc",
                                 name=f"sc{sb}_{tt}")
                    nc.tensor.matmul(sc[:, 0:SB],
                                     KT[0:DK, ts(tt, 128)],
                                     QT[0:DK, ts(sb, SB)],
                                     start=True, stop=True)
                    nc.tensor.matmul(sc[:, SB:2 * SB],
                                     KT[DK:PC, ts(tt, 128)],
                                     QT[DK:PC, ts(sb, SB)],
                                     start=True, stop=True)
                    sc_of[idx] = sc

                def emit_exp(idx):
                    sb, tt = items[idx]
                    et = ep.tile([128, 2 * SB], BF, tag="et",
                                 name=f"et{sb}_{tt}")
                    nc.scalar.activation(
                        et[:], sc_of.pop(idx)[:],
                        mybir.ActivationFunctionType.Exp, scale=0.125)
                    et_of[idx] = et

                def emit_ctx(idx, cur_iter):
                    sb, tt = items[idx]
                    if tt == 0:
                        ctx_of[sb] = tuple(
                            ps.tile([128, SB], F32, tag="ctx", bufs=4,
                                    name=f"c{n}_{sb}")
                            for n in ("A0", "B0", "A1", "B1"))
                    cA0, cB0, cA1, cB1 = ctx_of[sb]
                    et = et_of.pop(idx)
                    st_, sp_ = (tt == 0), (tt == NT - 1)
                    base = tt * VW
                    nc.tensor.matmul(cA0[0:AUG, :],
                                     VnA[0:DK, base:base + AUG],
                                     et[0:DK, 0:SB], start=st_, stop=sp_)
                    nc.tensor.matmul(cB0[0:AUG, :],
                                     VnA[DK:PC, base:base + AUG],
                                     et[DK:PC, 0:SB], start=st_, stop=sp_)
                    nc.tensor.matmul(cA1[0:AUG, :],
                                     VnA[0:DK, base + DK + 1:base + VW],
                                     et[0:DK, SB:2 * SB],
                                     start=st_, stop=sp_)
                    nc.tensor.matmul(cB1[0:AUG, :],
                                     VnA[DK:PC, base + DK + 1:base + VW],
                                     et[DK:PC, SB:2 * SB],
                                     start=st_, stop=sp_)
                    if sp_:
                        emit_epilogue(sb, cur_iter)

                def emit_epilogue(sb, cur_iter):
                    cA0, cB0, cA1, cB1 = ctx_of.pop(sb)
                    # combine halves into SBUF (frees the ctx PSUM ring for
                    # the next block after just these two DVE adds), then
                    # fast-reciprocal of the denominators
                    cu0 = cup.tile([AUG, SB], F32, tag="cu", bufs=4,
                                   name=f"cu0_{sb}")
                    cu1 = cup.tile([AUG, SB], F32, tag="cu", bufs=4,
                                   name=f"cu1_{sb}")
                    nc.vector.tensor_copy(cu0[:], cA0[0:AUG, :])
                    nc.vector.tensor_copy(cu1[:], cA1[0:AUG, :])
                    nc.vector.tensor_add(cu0[:], cu0[:], cB0[0:AUG, :])
                    nc.vector.tensor_add(cu1[:], cu1[:], cB1[0:AUG, :])
                    nc.vector.reciprocal(rdenf[0:1, :], cu0[DK:AUG, :])
                    nc.vector.reciprocal(rdenf[32:33, :], cu1[DK:AUG, :])
                    nc.vector.tensor_copy(rdenb[0:1, :], rdenf[0:1, :])
                    nc.vector.tensor_copy(rdenb[32:33, :], rdenf[32:33, :])

                    def norm():
                        bg = ps.tile([128, 2 * SB], F32, tag="sc",
                                     name=f"bps{sb}")
                        bpsv = bg[:, 0:SB]
                        nc.tensor.matmul(bpsv, e01[:, :], rdenb[:, :],
                                         start=True, stop=True)
                        ctxT = cxp.tile([128, SB], BF, tag="ctxT",
                                        name=f"ctxT{sb}")
                        nc.vector.tensor_mul(ctxT[0:DK, :], cu0[0:DK, :],
                                             bpsv[0:DK, :])
                        nc.vector.tensor_mul(ctxT[DK:PC, :], cu1[0:DK, :],
                                             bpsv[DK:PC, :])
                        ctxT_of[sb] = ctxT
                    at(cur_iter + 10, norm)

                    def po_piece(j, sb=sb):
                        st_c, nh = j // 2, j % 2
                        ctxT = ctxT_of[sb]
                        pg = ps.tile([128, 2 * SB], F32, tag="sc",
                                     name=f"po{sb}_{j}")
                        poA, poB = pg[:, 0:SB], pg[:, SB:2 * SB]
                        nc.tensor.matmul(poA,
                                         ctxT[0:DK, ts(st_c, 128)],
                                         w_o[0:DK, ts(nh, SB)],
                                         start=True, stop=True)
                        nc.tensor.matmul(poB,
                                         ctxT[DK:PC, ts(st_c, 128)],
                                         w_o[DK:PC, ts(nh, SB)],
                                         start=True, stop=True)
                        ot = otp.tile([128, SB], F32, tag="ot",
                                      name=f"ot{sb}_{j}")
                        p_s = scr.tile([128, 2 * SB], F32, tag="scr",
                                       name=f"pos{sb}_{j}")
                        nc.vector.tensor_copy(p_s[:, 0:SB], poA)
                        nc.vector.tensor_add(ot[:], p_s[:, 0:SB], poB)
                        nc.gpsimd.dma_start(
                            out[sb * SB + st_c * 128:
                                sb * SB + (st_c + 1) * 128,
                                ts(nh, SB)], ot[:])
                    for j in range(8):
                        at(cur_iter + 12 + 2 * j, lambda j=j: po_piece(j))

                # schedule block-0 K-proj tail (group j feeds scores(4j)
                # at iter 4j-1; thunk iters track the DMA arrival pacing) +
                # V-natural production (v tiles land late: lag the thunks so
                # the in-order PE never blocks on a v DMA)
                KP_ITER = {1: 1, 2: 3, 3: 5, 4: 9, 5: 13, 6: 17, 7: 21}
                for j, it in KP_ITER.items():
                    at(it, lambda j=j: kproj(j))
                VN_ITER = {}
                for tt in range(NT):
                    g, k = tt // 8, tt % 8
                    VN_ITER[tt] = (6, 20, 25, 30)[g] + k
                    at(VN_ITER[tt], lambda tt=tt: vnat(tt))
                for p in (1, 2, 3):
                    # DMA for pair p early in block 2p-2, proj during 2p-1
                    # (quarter-width atomic thunks so the borrowed PSUM slot
                    # is held only ~1.8us and ACT never starves)
                    at((2 * p - 2) * NT + 1,
                       lambda p=p: qtiles.__setitem__(p, q_dma(p)))
                    at((2 * p - 1) * NT + 8,
                       lambda p=p: qproj(2 * p, 0, 256))
                    at((2 * p - 1) * NT + 11,
                       lambda p=p: qproj(2 * p, 1, 256))
                    at((2 * p - 1) * NT + 20,
                       lambda p=p: qproj(2 * p + 1, 0, 256))
                    at((2 * p - 1) * NT + 23,
                       lambda p=p: qproj(2 * p + 1, 1, 256))

                emit_sc(0)
                for i in range(n_items):
                    emit_exp(i)
                    if i + 1 < n_items:
                        emit_sc(i + 1)
                    for fn in thunks.pop(i, ()):
                        fn()
                    sb, tt = items[i]
                    if sb == 0:
                        sched = max(i + 6, VN_ITER[tt] + 2)
                    else:
                        sched = i + 4
                    if ctx_queue:
                        sched = max(sched, ctx_queue[-1][0])
                    ctx_queue.append((sched, i))
                    while ctx_queue and ctx_queue[0][0] <= i:
                        _, idx = ctx_queue.pop(0)
                        emit_ctx(idx, i)
                # drain remaining ctx + thunks
                i = n_items
                while ctx_queue or thunks:
                    for fn in thunks.pop(i, ()):
                        fn()
                    while ctx_queue and ctx_queue[0][0] <= i:
                        _, idx = ctx_queue.pop(0)
                        emit_ctx(idx, i)
                    i += 1
                    assert i < n_items + 200, "drain did not converge"

    return nc


_NC = None


def _get_nc():
    global _NC
    if _NC is None:
        _NC = _build()
        _split_multi_waits(_NC)
    return _NC


def kernel(q, k, v, Wq, bq, Wk, bk, Wv, bv, Wo, bo):
    global LAST_RESULT
    nc = _get_nc()

    q2, k2, v2 = (np.asarray(x, np.float32)[0] for x in (q, k, v))
    qTh = np.ascontiguousarray(q2.T).astype(BF16)
    kTh = np.ascontiguousarray(k2.T).astype(BF16)
    vTh = np.ascontiguousarray(v2.T).astype(BF16)

    in_maps = []
    for c in range(N_CORES):
        sl = slice(c * PC, (c + 1) * PC)
        in_maps.append({
            "qT": qTh, "kT": kTh, "vT": vTh,
            "wq": np.ascontiguousarray(np.asarray(Wq, np.float32)[sl].T).astype(BF16),
            "wk": np.ascontiguousarray(np.asarray(Wk, np.float32)[sl].T).astype(BF16),
            "wv": np.ascontiguousarray(np.asarray(Wv, np.float32)[sl].T).astype(BF16),
            "bqd": np.asarray(bq, np.float32)[sl].reshape(PC, 1).copy(),
            "bkd": np.asarray(bk, np.float32)[sl].reshape(PC, 1).copy(),
            "bvr": np.asarray(bv, np.float32)[sl].reshape(1, PC).astype(BF16),
            "wo": np.ascontiguousarray(np.asarray(Wo, np.float32)[:, sl].T).astype(BF16),
        })

    res = run_bass_kernel_spmd(nc, in_maps, core_ids=list(range(N_CORES)))
    LAST_RESULT = res

    acc = np.zeros((S, D), np.float32)
    for c in range(N_CORES):
        acc += res.results[c]["out"]
    acc += np.asarray(bo, np.float32)[None, :]
    return acc[None].astype(np.float32)
